# revision 12
# baseline (speedup 1.0000x reference)
"""GAT message-passing kernel for Trainium2, 8 NeuronCores.

Problem (see harness reference): for each head h:
    Wh   = x @ W[h]                                  [B,N,F]
    e    = leaky_relu((Wh@a_src)[:,:,None] + (Wh@a_dst)[:,None,:], 0.2)
    att  = exp(where(adj>0, e, -9e15)) * big_w        [B,N,N]
    att /= clip(sum(att, axis=1), 1e-12)              (column L1 norm)
    out_h = elu(att @ Wh)
    out   = concat over heads                         [B,N,H*F]

big_w is bipartite: nonzero only on blocks (i<U, j>=U) [= weights.T] and
(i>=U, j<U) [= weights]. So att has only two 1024x1024 nonzero blocks and
the column normalizer of block-A columns is fully determined by block-A
rows (and likewise for block B).

Sharding: core c -> (b = c//4, X = (c%4)//2, hg = c%2): each core owns the
1024 output rows of one bipartite block for one batch and computes two of
the four heads for those rows. Denominators are local to a core (no
collectives); host gathers disjoint output slabs.

All layout work happens on the host: the adjacency mask is staged as a
0/-30000 additive log-mask and the block weights as ln(w), both
pre-transposed to the [j, i] layout and bf16; xT = x.T with the core's
own rows first, so the device program is uniform SPMD with zero
transposes. The attention entry is then
    att = exp(leaky_relu(s_i + d_j) + adjL + ln w)
so a tile needs only: Prelu (Act), one bf16 tensor-tensor add of the
precombined log-mask (DVE/GpSimd at 2x), and Exp on Act whose accum_out
emits the column denominator for free. A second route computes the
leaky_relu as max(u, alpha*u) on DVE tensor_scalar (4x) ops to balance
Act vs DVE; tiles are assigned to routes/engines by the knobs below.

att tiles live in [j, i] layout so they are directly the rhs of
transposed-output matmuls (n=512) and out^T goes back bf16; the host
transposes to the final layout. elu(y) = max(y, exp(min(y,0)) - 1).
"""

import threading
import numpy as np

B, N, FIN, F, H, U = 2, 2048, 128, 128, 4, 1024
P = 128
JT = U // P            # 8 tiles along the contraction (column) axis
NH = 2                 # heads per core
ALPHA = 0.2
NEGL = -30000.0        # additive log-mask for adj == 0

# Route/engine knobs: D_TILES computes leaky_relu on DVE (tensor_scalar
# 4x ops) instead of Act Prelu; Z_GPSIMD runs the log-mask add of
# Prelu-route tiles on GpSimd instead of DVE.
D_TILES = {
    (0, 4), (0, 5), (0, 6), (0, 7),
    (1, 2), (1, 3), (1, 5), (1, 6), (1, 7),
}
Z_GPSIMD = {(0, 0), (0, 1), (0, 2), (0, 3), (1, 0), (1, 1), (1, 4)}

TRACE = False          # set by test.py for profiling runs
LAST_EXEC_NS = None    # exec_time_ns of the last traced run
_BUILD_LOCK = threading.Lock()
_CACHE = {}


def _build_program():
    from concourse import bacc
    import concourse.mybir as mybir
    import concourse.tile as tile

    dt = mybir.dt
    Alu = mybir.AluOpType
    Act = mybir.ActivationFunctionType

    nc = bacc.Bacc("TRN2", target_bir_lowering=False, debug=False, num_devices=8)

    adjL = nc.dram_tensor("adjL", [U, U], dt.bfloat16, kind="ExternalInput")
    wLn = nc.dram_tensor("wLn", [U, U], dt.bfloat16, kind="ExternalInput")
    xT = nc.dram_tensor("xT", [FIN, N], dt.bfloat16, kind="ExternalInput")
    wpar = nc.dram_tensor("wpar", [FIN, NH, F], dt.bfloat16, kind="ExternalInput")
    asrc = nc.dram_tensor("asrc", [F, NH], dt.bfloat16, kind="ExternalInput")
    adstr = nc.dram_tensor("adstr", [1, NH * F], dt.float32, kind="ExternalInput")
    outT = nc.dram_tensor("outT", [NH * F, U], dt.bfloat16, kind="ExternalOutput")

    with tile.TileContext(nc) as tc:
        with (
            tc.tile_pool(name="persist", bufs=1) as persist,
            tc.tile_pool(name="lr", bufs=3) as lr_pool,
            tc.tile_pool(name="zz", bufs=3) as zz_pool,
            tc.tile_pool(name="uu", bufs=2) as uu_pool,
            tc.tile_pool(name="elu", bufs=2) as elu_pool,
            tc.tile_pool(name="tr", bufs=2) as tr_pool,
            tc.tile_pool(name="ps_big", bufs=4, space="PSUM") as ps_big,
            tc.tile_pool(name="ps_sml", bufs=2, space="PSUM") as ps_sml,
        ):
            # ---------------- loads (small first, then xT chunks, then mask)
            wpar_sb = persist.tile([P, NH, F], dt.bfloat16)
            nc.scalar.dma_start(out=wpar_sb, in_=wpar[:, :, :])
            asrc_sb = persist.tile([P, NH], dt.bfloat16)
            nc.scalar.dma_start(out=asrc_sb, in_=asrc[:, :])
            adstr_sb = persist.tile([1, NH * F], dt.float32)
            nc.scalar.dma_start(out=adstr_sb, in_=adstr[:, :])
            xT_sb = persist.tile([P, N], dt.bfloat16)
            for q in range(4):
                nc.scalar.dma_start(
                    out=xT_sb[:, q * 512 : (q + 1) * 512],
                    in_=xT[:, q * 512 : (q + 1) * 512],
                )
            wLn_sb = persist.tile([P, JT, U], dt.bfloat16)
            adjL_sb = persist.tile([P, JT, U], dt.bfloat16)
            for jt in range(JT):
                nc.scalar.dma_start(
                    out=wLn_sb[:, jt, :], in_=wLn[jt * P : (jt + 1) * P, :]
                )
                nc.sync.dma_start(
                    out=adjL_sb[:, jt, :], in_=adjL[jt * P : (jt + 1) * P, :]
                )

            # ---------------- feature path
            # whT (own rows only, for s); wh_j rows for contraction nodes.
            whT = persist.tile([P, NH, U], dt.bfloat16)
            wh_j = persist.tile([P, NH * JT, F], dt.float32)
            s_row = [persist.tile([1, U], dt.bfloat16, name=f"s_row{h}") for h in range(NH)]
            s_bc = [persist.tile([P, U], dt.bfloat16, name=f"s_bc{h}") for h in range(NH)]
            d_cols = [persist.tile([P, JT], dt.float32, name=f"d_cols{h}") for h in range(NH)]

            adst_bc = persist.tile([P, NH * F], dt.float32)
            nc.gpsimd.partition_broadcast(adst_bc, adstr_sb)

            for h in range(NH):
                for q in range(2):
                    wt_ps = ps_big.tile([P, 512], dt.float32, tag="big", name=f"wt_ps{h}{q}")
                    nc.tensor.matmul(
                        wt_ps,
                        wpar_sb[:, h, :],
                        xT_sb[:, q * 512 : (q + 1) * 512],
                        start=True,
                        stop=True,
                    )
                    nc.scalar.copy(whT[:, h, q * 512 : (q + 1) * 512], wt_ps)
                for q in range(2):
                    s_ps = ps_sml.tile([1, 512], dt.float32, tag="s", name=f"s_ps{h}{q}")
                    nc.tensor.matmul(
                        s_ps,
                        asrc_sb[:, h : h + 1],
                        whT[:, h, q * 512 : (q + 1) * 512],
                        start=True,
                        stop=True,
                    )
                    nc.vector.tensor_copy(s_row[h][:, q * 512 : (q + 1) * 512], s_ps)
                nc.gpsimd.partition_broadcast(s_bc[h], s_row[h])
                for g in range(2):
                    wj_ps = ps_big.tile([P, 512], dt.float32, tag="big", name=f"wj_ps{h}{g}")
                    for k in range(4):
                        nc.tensor.matmul(
                            wj_ps[:, k * P : (k + 1) * P],
                            xT_sb[:, U + (4 * g + k) * P : U + (4 * g + k + 1) * P],
                            wpar_sb[:, h, :],
                            start=True,
                            stop=True,
                        )
                    nc.vector.tensor_copy(
                        wh_j[:, h * JT + 4 * g : h * JT + 4 * g + 4, :],
                        wj_ps.rearrange("p (a b) -> p a b", a=4),
                    )
                # d[j] = Wh[j] . a_dst via free-axis accum on gpsimd
                for jt in range(JT):
                    d_tr = tr_pool.tile([P, F], dt.bfloat16, tag="tr")
                    nc.vector.scalar_tensor_tensor(
                        out=d_tr,
                        in0=wh_j[:, h * JT + jt, :],
                        scalar=1.0,
                        in1=adst_bc[:, h * F : (h + 1) * F],
                        op0=Alu.mult,
                        op1=Alu.mult,
                        accum_out=d_cols[h][:, jt : jt + 1],
                    )

            # ---------------- combined log-mask lnM = adjL + ln(w)
            lnM = persist.tile([P, JT, U], dt.bfloat16)
            for jt in range(JT):
                nc.vector.tensor_tensor(
                    lnM[:, jt, :], adjL_sb[:, jt, :], wLn_sb[:, jt, :], Alu.add
                )

            # ---------------- attention + output, head/group-pipelined
            att = persist.tile([P, NH * JT, U], dt.bfloat16)
            den = [persist.tile([P, JT], dt.float32, name=f"den{h}") for h in range(NH)]
            rec = [persist.tile([P, JT], dt.float32, name=f"rec{h}") for h in range(NH)]
            whs = persist.tile([P, NH * JT, F], dt.bfloat16)
            outT_sb = persist.tile([P, NH, U], dt.bfloat16)
            out_ps = {}

            def att_tile(h, jt):
                if (h, jt) in D_TILES:
                    # leaky_relu on DVE: u = s+d; lr = max(u, alpha*u)
                    u = uu_pool.tile([P, U], dt.bfloat16, tag="u")
                    nc.vector.tensor_scalar(
                        out=u,
                        in0=s_bc[h],
                        scalar1=d_cols[h][:, jt : jt + 1],
                        scalar2=None,
                        op0=Alu.add,
                    )
                    ua = uu_pool.tile([P, U], dt.bfloat16, tag="ua")
                    nc.vector.tensor_scalar(
                        out=ua, in0=u, scalar1=ALPHA, scalar2=None, op0=Alu.mult
                    )
                    lr = lr_pool.tile([P, U], dt.bfloat16, tag="lr")
                    nc.vector.tensor_tensor(lr, u, ua, Alu.max)
                else:
                    lr = lr_pool.tile([P, U], dt.bfloat16, tag="lr")
                    nc.scalar.activation(
                        lr,
                        s_bc[h],
                        Act.Prelu,
                        bias=d_cols[h][:, jt : jt + 1],
                        scale=1.0,
                        alpha=ALPHA,
                    )
                z = zz_pool.tile([P, U], dt.bfloat16, tag="z")
                zeng = nc.gpsimd if (h, jt) in Z_GPSIMD else nc.vector
                zeng.tensor_tensor(z, lr, lnM[:, jt, :], Alu.add)
                nc.scalar.activation(
                    att[:, h * JT + jt, :],
                    z,
                    Act.Exp,
                    accum_out=den[h][:, jt : jt + 1],
                )

            def group(h, g):
                for jt in range(4 * g, 4 * g + 4):
                    att_tile(h, jt)
                sl = slice(4 * g, 4 * g + 4)
                nc.vector.tensor_scalar(
                    out=rec[h][:, sl],
                    in0=den[h][:, sl],
                    scalar1=1e-12,
                    scalar2=None,
                    op0=Alu.max,
                )
                nc.vector.reciprocal(rec[h][:, sl], rec[h][:, sl])
                for jt in range(4 * g, 4 * g + 4):
                    nc.vector.tensor_scalar(
                        out=whs[:, h * JT + jt, :],
                        in0=wh_j[:, h * JT + jt, :],
                        scalar1=rec[h][:, jt : jt + 1],
                        scalar2=None,
                        op0=Alu.mult,
                    )
                for q in range(2):
                    if g == 0:
                        out_ps[(h, q)] = ps_big.tile(
                            [P, 512], dt.float32, tag="big", name=f"o_ps{h}{q}"
                        )
                    o_ps = out_ps[(h, q)]
                    for jt in range(4 * g, 4 * g + 4):
                        nc.tensor.matmul(
                            o_ps,
                            whs[:, h * JT + jt, :],
                            att[:, h * JT + jt, q * 512 : (q + 1) * 512],
                            start=(jt == 0),
                            stop=(jt == JT - 1),
                        )

            def out_elu(h):
                # elu(y) = max(y, exp(min(y, 0)) - 1)
                for q in range(2):
                    o_ps = out_ps[(h, q)]
                    m = elu_pool.tile([P, 512], dt.float32, tag="m")
                    nc.vector.tensor_scalar(
                        out=m, in0=o_ps, scalar1=0.0, scalar2=None, op0=Alu.min
                    )
                    q2 = elu_pool.tile([P, 512], dt.float32, tag="q2")
                    nc.scalar.activation(q2, m, Act.Exp)
                    nc.vector.scalar_tensor_tensor(
                        out=outT_sb[:, h, q * 512 : (q + 1) * 512],
                        in0=q2,
                        scalar=1.0,
                        in1=o_ps,
                        op0=Alu.subtract,
                        op1=Alu.max,
                    )
                nc.sync.dma_start(
                    out=outT[h * P : (h + 1) * P, :], in_=outT_sb[:, h, :]
                )

            group(0, 0)
            group(0, 1)
            group(1, 0)
            out_elu(0)
            group(1, 1)
            out_elu(1)

    nc.compile()
    return nc


def _stage_inputs(x, weights, W, a, adj):
    import ml_dtypes

    bf16 = ml_dtypes.bfloat16
    per_bx = {}
    for b in range(B):
        lw = np.where(weights[b] > 0, np.log(np.maximum(weights[b], 1e-38)), NEGL)
        for X in range(2):
            if X == 0:
                adjl = np.where(adj[b, :U, U:].T > 0, 0.0, NEGL).astype(bf16)
                wln = lw.astype(bf16)
                own, other = x[b, :U], x[b, U:]
            else:
                adjl = np.where(adj[b, U:, :U].T > 0, 0.0, NEGL).astype(bf16)
                wln = np.ascontiguousarray(lw.T).astype(bf16)
                own, other = x[b, U:], x[b, :U]
            xt = np.ascontiguousarray(
                np.concatenate([own, other], axis=0).T
            ).astype(bf16)
            per_bx[(b, X)] = (adjl, wln, xt)

    per_hg = {}
    for hg in range(2):
        wp = np.ascontiguousarray(
            np.transpose(W[2 * hg : 2 * hg + 2], (1, 0, 2))
        ).astype(bf16)
        asr = np.ascontiguousarray(a[2 * hg : 2 * hg + 2, :F, 0].T).astype(bf16)
        ads = np.ascontiguousarray(
            a[2 * hg : 2 * hg + 2, F:, 0].reshape(1, NH * F)
        ).astype(np.float32)
        per_hg[hg] = (wp, asr, ads)

    in_maps = []
    for c in range(8):
        b, X, hg = c // 4, (c % 4) // 2, c % 2
        adjl, wln, xt = per_bx[(b, X)]
        wp, asr, ads = per_hg[hg]
        in_maps.append(
            {"adjL": adjl, "wLn": wln, "xT": xt, "wpar": wp, "asrc": asr, "adstr": ads}
        )
    return in_maps


def kernel(x, weights, W, a, adj):
    global LAST_EXEC_NS
    from concourse.bass_utils import run_bass_kernel_spmd

    x = np.asarray(x, dtype=np.float32)
    weights = np.asarray(weights, dtype=np.float32)
    W = np.asarray(W, dtype=np.float32)
    a = np.asarray(a, dtype=np.float32)
    adj = np.asarray(adj, dtype=np.int32)

    with _BUILD_LOCK:
        if "nc" not in _CACHE:
            _CACHE["nc"] = _build_program()
    nc = _CACHE["nc"]

    in_maps = _stage_inputs(x, weights, W, a, adj)
    res = run_bass_kernel_spmd(nc, in_maps, core_ids=list(range(8)), trace=TRACE)
    if res.exec_time_ns is not None:
        LAST_EXEC_NS = res.exec_time_ns

    out = np.empty((B, N, H * F), dtype=np.float32)
    for c in range(8):
        b, X, hg = c // 4, (c % 4) // 2, c % 2
        ot = np.asarray(res.results[c]["outT"]).astype(np.float32)  # [2F, U]
        out[b, X * U : (X + 1) * U, hg * 2 * F : (hg + 1) * 2 * F] = ot.T
    return out
